# revision 4
# baseline (speedup 1.0000x reference)
"""Trainium2 Bass kernel for nn_BinaryLoss (BCE triangle-mesh loss).

Structure
---------
Host (integer combinatorics on the tiny index tensors only): sorted-triangle
key table -> unique keys; undirected GT edge set; per-vertex unique-triangle
counts; candidate-triple membership gt_mask [N,256]; manifold row mask w [N];
edge mask gm [N,16].  Identities used:
  * gt_labels_masked == gt_mask,
  * softplus(x) = -ln(sigmoid(-x)), so every loss term is a log of a product
    of sigmoids.  The device ships segmented PRODUCTS of sigmoids; the host
    takes logs of the (few) partial products and does the scalar reduction.
    Only the Sigmoid activation is used on device -> a single ACT table load.

Device (8 cores data-parallel, per core, all logit math in fp16):
  * s = sigmoid(-gsel) maps "2nd/3rd largest logit" to "2nd/3rd SMALLEST
    sigmoid" where fp16 has fine relative precision, so a 7-instruction
    DVE chain (segmin -> is_equal mask -> +2 push -> segmin -> ...) extracts
    exact rank-2/3 values per 16-wide group for all 6272 groups at once.
  * sp(-p2) = -ln(1 - m2), sp(p3) = -ln(m3): segmented products of (1-m2)
    and m3 (and of sigmoid(-x) over selected rows) ship as [128,48] f32.
  * Dummy heartbeat DMAs ride the DVE chain to keep the HW DMA engines out
    of their slow idle-poll state so the final output DMA completes fast.
Pad rows use -15 (sigmoid -> 1.0 exactly, neutral in products); pad groups
[+8, +7, -15 x14] keep distinct fp16 top-2 so the masked-min chain yields
neutral (exactly 1.0) pos/neg terms.
"""
import numpy as np

N_CORES = 8
B_PAD = 15.0
NROW = 128           # selected rows per core (one per partition)
M = 256              # logits per row
G = 49               # groups per partition per core
GPC = G * 128        # groups per core
L = 8                # max gt_mask nonzeros per row


# ---------------------------------------------------------------- host prep
def _host_prep(pred_logits, points, knn_indices, gt_triangles):
    N, K = knn_indices.shape
    m = (K - 1) * (K - 1)
    num_pts = points.shape[0]
    P = num_pts + 1

    tri = np.sort(np.asarray(gt_triangles, dtype=np.int64), axis=1)
    keys = tri[:, 0] * (P * P) + tri[:, 1] * P + tri[:, 2]
    uk = np.unique(keys)

    ut0, ut1, ut2 = uk // (P * P), (uk // P) % P, uk % P
    counts = np.zeros(P, np.float64)
    np.add.at(counts, ut0, 1.0)
    np.add.at(counts, ut1, (ut1 != ut0).astype(np.float64))
    np.add.at(counts, ut2, (ut2 != ut1).astype(np.float64))
    all_N_gt = counts[np.asarray(knn_indices[:, 0], dtype=np.int64)]

    e_u = np.concatenate([np.minimum(tri[:, 0], tri[:, 1]),
                          np.minimum(tri[:, 1], tri[:, 2]),
                          np.minimum(tri[:, 0], tri[:, 2])])
    e_v = np.concatenate([np.maximum(tri[:, 0], tri[:, 1]),
                          np.maximum(tri[:, 1], tri[:, 2]),
                          np.maximum(tri[:, 0], tri[:, 2])])
    ekeys = np.unique(e_u * P + e_v)

    c = np.asarray(knn_indices[:, 0], dtype=np.int64)[:, None]
    a = np.asarray(knn_indices[:, 1:], dtype=np.int64)
    q = np.minimum(c, a) * P + np.maximum(c, a)
    pos = np.clip(np.searchsorted(ekeys, q.ravel()), 0, len(ekeys) - 1)
    gm = (ekeys[pos] == q.ravel()).reshape(N, K - 1)

    e0 = np.repeat(a, K - 1, axis=1)
    e1 = np.tile(a, (1, K - 1))
    v0 = np.broadcast_to(c, e0.shape)
    cand = np.stack([v0, e0, e1], axis=-1)
    cand.sort(axis=-1)
    ck = cand[..., 0] * (P * P) + cand[..., 1] * P + cand[..., 2]
    cpos = np.clip(np.searchsorted(uk, ck.ravel()), 0, len(uk) - 1)
    gt_mask = (uk[cpos] == ck.ravel()).reshape(N, m)

    all_N_pred = gt_mask.sum(1).astype(np.float64)
    manifold = (all_N_gt * 2.0) == all_N_pred
    w = manifold.astype(np.float32)

    inv_denom = 1.0 / max(float(w.sum(dtype=np.float64)) * m, 1.0)
    inv_cnt = 1.0 / max(float(gm.sum(dtype=np.float64)), 1.0)
    return gt_mask, gm, w, inv_denom, inv_cnt


def _make_shards(x, gt_mask, gm, w):
    """Per-core input dicts. One fused fp16 tensor [128, G*16 + M + L]."""
    N = x.shape[0]
    f16 = np.float16

    # selected (manifold) rows, padded to 8*128
    sel = np.nonzero(w)[0]
    CAP = NROW * N_CORES
    assert len(sel) <= CAP, (len(sel), CAP)
    xs = np.full((CAP, M), -B_PAD, np.float32)
    xs[:len(sel)] = x[sel]

    # masked-x values per selected row, padded to L
    rr, cc = np.nonzero(gt_mask[sel])
    row_starts = np.zeros(CAP + 1, np.int64)
    np.add.at(row_starts, rr + 1, 1)
    row_starts = np.cumsum(row_starts)
    ranks = np.arange(len(rr)) - row_starts[rr]
    assert ranks.max(initial=0) < L
    xmv = np.zeros((CAP, L), np.float32)
    xmv[rr, ranks] = xs[rr, cc]

    # compacted gm groups
    gn, gi = np.nonzero(gm)
    total = len(gn)
    assert total <= GPC * N_CORES, total
    pl3 = x.reshape(N, 16, 16)
    pad_group = np.full(16, -B_PAD, np.float32)
    pad_group[0] = 8.0
    pad_group[1] = 7.0
    groups = np.broadcast_to(pad_group, (GPC * N_CORES, 16)).copy()
    groups[:total] = pl3[gn, gi, :]

    in_maps = []
    for core in range(N_CORES):
        gsl = groups[core * GPC:(core + 1) * GPC]
        gsl = np.ascontiguousarray(
            gsl.reshape(G, 128, 16).transpose(1, 0, 2)).reshape(128, G * 16)
        r0 = core * NROW
        fused = np.concatenate(
            [gsl, xs[r0:r0 + NROW], xmv[r0:r0 + NROW]], axis=1).astype(f16)
        in_maps.append({"inp": np.ascontiguousarray(fused)})
    return in_maps


# ---------------------------------------------------------------- bass build
def _build_bass():
    from contextlib import ExitStack

    import concourse.bacc as bacc
    import concourse.mybir as mybir
    import concourse.tile as tile

    f32 = mybir.dt.float32
    f16 = mybir.dt.float16
    AFT = mybir.ActivationFunctionType
    ALU = mybir.AluOpType
    AX = mybir.AxisListType

    G16 = G * 16
    TOT = G16 + M + L          # fused input columns
    SPSEG = M // 8             # 32 sigmoid-product segments per row

    nc = bacc.Bacc(
        "TRN2", target_bir_lowering=False, debug=False,
        enable_asserts=False, num_devices=N_CORES,
    )
    in_d = nc.dram_tensor("inp", [128, TOT], f16, kind="ExternalInput").ap()
    out_d = nc.dram_tensor("out", [128, 48], f32, kind="ExternalOutput").ap()

    with tile.TileContext(nc) as tc, ExitStack() as ctx:
        from concourse.tile import add_dep_helper

        def chain(lst):
            for a_, b_ in zip(lst, lst[1:]):
                add_dep_helper(b_.ins, a_.ins, sync=True, reason="engine order")

        pool = ctx.enter_context(tc.tile_pool(name="main", bufs=1))

        inp = pool.tile([128, TOT], f16)
        dma_g = nc.sync.dma_start(inp[:, :G16], in_d[:, :G16])
        dma_x = nc.sync.dma_start(inp[:, G16:], in_d[:, G16:])

        acts = []
        # s = sigmoid(-gsel)
        sgs = pool.tile([128, G16], f16)
        acts.append(nc.scalar.activation(sgs[:], inp[:, :G16], AFT.Sigmoid,
                                         scale=-1.0))
        # sigmoid(-x) over selected rows
        sgx = pool.tile([128, M], f16)
        acts.append(nc.scalar.activation(sgx[:], inp[:, G16:G16 + M],
                                         AFT.Sigmoid, scale=-1.0))

        out_t = pool.tile([128, 48], f32)
        # xm sum via activation accumulate (Identity); out scratch unused
        xm_scr = pool.tile([128, L], f32)
        acts.append(nc.scalar.activation(xm_scr[:], inp[:, G16 + M:],
                                         AFT.Identity,
                                         accum_out=out_t[:, 46:47]))
        nc.vector.memset(out_t[:, 47:48], 0.0)

        # ---- DVE masked bottom-3 chain over [128, G, 16] fp16 ----
        s3d = sgs[:].rearrange("p (g e) -> p g e", e=16)
        m1 = pool.tile([128, G], f16)
        m1done = nc.vector.tensor_reduce(m1[:], s3d, axis=AX.X, op=ALU.min)
        m1b = m1[:].unsqueeze(-1).broadcast_to([128, G, 16])

        e1 = pool.tile([128, G16], f16)
        e1_3 = e1[:].rearrange("p (g e) -> p g e", e=16)
        nc.vector.scalar_tensor_tensor(e1_3, s3d, 1.0, m1b,
                                       op0=ALU.mult, op1=ALU.is_equal)
        s2 = pool.tile([128, G16], f16)
        s2_3 = s2[:].rearrange("p (g e) -> p g e", e=16)
        nc.vector.scalar_tensor_tensor(s2_3, e1_3, 2.0, s3d,
                                       op0=ALU.mult, op1=ALU.add)
        m2 = pool.tile([128, G], f16)
        nc.vector.tensor_reduce(m2[:], s2_3, axis=AX.X, op=ALU.min)
        m2b = m2[:].unsqueeze(-1).broadcast_to([128, G, 16])

        e2 = pool.tile([128, G16], f16)
        e2_3 = e2[:].rearrange("p (g e) -> p g e", e=16)
        nc.vector.scalar_tensor_tensor(e2_3, s2_3, 1.0, m2b,
                                       op0=ALU.mult, op1=ALU.is_equal)
        s3t = pool.tile([128, G16], f16)
        s3_3 = s3t[:].rearrange("p (g e) -> p g e", e=16)
        nc.vector.scalar_tensor_tensor(s3_3, e2_3, 2.0, s2_3,
                                       op0=ALU.mult, op1=ALU.add)
        m3 = pool.tile([128, G], f16)
        m3done = nc.vector.tensor_reduce(m3[:], s3_3, axis=AX.X, op=ALU.min)

        # pos term = 1 - m2 on ScalarE
        post = pool.tile([128, G], f16)
        acts.append(nc.scalar.activation(post[:], m2[:], AFT.Identity,
                                         scale=-1.0, bias=1.0))

        # ---- segmented products -> out tile ----
        nc.vector.tensor_reduce(
            out_t[:, 0:SPSEG], sgx[:].rearrange("p (k l) -> p k l", l=8),
            axis=AX.X, op=ALU.mult)
        nc.vector.tensor_reduce(
            out_t[:, 32:39], post[:].rearrange("p (k l) -> p k l", l=7),
            axis=AX.X, op=ALU.mult)
        nc.vector.tensor_reduce(
            out_t[:, 39:46], m3[:].rearrange("p (k l) -> p k l", l=7),
            axis=AX.X, op=ALU.mult)
        nc.sync.dma_start(out_d[:], out_t[:], single_packet=True)

        # heartbeat DMAs keep the HW DMA engines awake through the DVE chain
        hb = pool.tile([128, 4], f16)
        hb1 = nc.sync.dma_start(hb[:, 0:2], in_d[:, 0:2])
        add_dep_helper(hb1.ins, m1done.ins, sync=True, reason="hb spacing")
        hb2 = nc.sync.dma_start(hb[:, 2:4], in_d[:, 0:2])
        add_dep_helper(hb2.ins, m3done.ins, sync=True, reason="hb spacing")

        chain(acts)

    nc.compile()
    return nc


# ---------------------------------------------------------------- entrypoint
def _run(pred_logits, points, knn_indices, gt_triangles, **run_kwargs):
    from concourse.bass_utils import run_bass_kernel_spmd

    x = np.ascontiguousarray(np.asarray(pred_logits, dtype=np.float32))
    gt_mask, gm, w, inv_denom, inv_cnt = _host_prep(
        pred_logits, points, knn_indices, gt_triangles)
    in_maps = _make_shards(x, gt_mask, gm, w)
    nc = _build_bass()
    res = run_bass_kernel_spmd(nc, in_maps, core_ids=list(range(N_CORES)),
                               **run_kwargs)
    S_sp = S_xm = S_pos = S_neg = 0.0
    for r in res.results:
        o = np.asarray(r["out"], dtype=np.float64).reshape(128, 48)
        S_sp += -np.log(o[:, 0:32]).sum()
        S_pos += -np.log(o[:, 32:39]).sum()
        S_neg += -np.log(o[:, 39:46]).sum()
        S_xm += o[:, 46].sum()
    total = np.array([(S_sp - S_xm) * inv_denom,
                      S_pos * inv_cnt,
                      S_neg * inv_cnt])
    return total.astype(np.float32), res


def kernel(pred_logits, points, knn_indices, gt_triangles):
    out, _ = _run(pred_logits, points, knn_indices, gt_triangles)
    return out
